# revision 20
# baseline (speedup 1.0000x reference)
"""Trainium2 Bass kernel for a 6-layer post-LN transformer encoder.

Sharding: data-parallel over batch — B=8, one batch element per NeuronCore,
no collectives.  Each core runs the full 6-layer encoder on its [S, D] slice.

Device-side layout: activations are kept feature-major ([D, S], "xT") in SBUF
so every matmul uses input-major weights as the stationary (lhsT) operand and
PE contracts over the partition dim:

  out[m, n] = sum_k lhsT[k, m] * rhs[k, n]

All matmul operands are bfloat16 (1 PE row/cycle, half the HBM/SBUF traffic of
fp32); PSUM accumulation stays fp32.  Attention is computed transposed
(scoresT[t, s]):

- scores for the two heads of a pair use K=64 stationaries at partition rows
  0-63 / 64-127, issued back-to-back into the two banks of one [128, 1024]
  PSUM tile -> the PE runs them concurrently (row-group tiling).
- one [128, 1024] Exp covers both heads of one (t, s-half) group, amortizing
  the ACT pipeline overhead; sc tiles are double-buffered so ACT stays fed.
- softmax denominators fall out of a ones-column appended to V (ctx matmul
  row 64); 1/denom via single-op reciprocal_approx_fast, broadcast to both
  head rows with one K=2 matmul against a [2,128] selector constant.
- ctx pairs are packed [head A rows 0-63 | head B rows 64-127] so Wo runs at
  K=128 (half the instructions of per-head K=64).

FFN weights stay SBUF-resident for the whole layer (loaded once, bf16), the
bias+ReLU runs fused on the Scalar engine, and LayerNorm's broadcast
subtract/multiply run on GpSimd to keep the Vector engine off the critical
path.
"""

import numpy as np

L, H, D, DK, DFF = 6, 8, 512, 64, 2048
B, S = 8, 1024
EPS = 1e-5
P = 128
NDT = D // P        # 4  d-tiles
NST = S // P        # 8  s/t-tiles
NFT = DFF // P      # 16 dff-tiles
NPAIR = H // 2      # 4  head pairs
NH = S // 512       # 2  s-halves (512-wide fp32-PSUM matmul free dim)
SCALE = 1.0 / np.sqrt(np.float32(DK))

_CACHE = {}


def _build_nc():
    import concourse.bass as bass
    import concourse.bacc as bacc
    import concourse.tile as tile
    from concourse import mybir

    fp32 = mybir.dt.float32
    fp32r = mybir.dt.float32r
    bf16 = mybir.dt.bfloat16
    i32 = mybir.dt.int32
    AF = mybir.ActivationFunctionType
    OP = mybir.AluOpType

    class _Bacc(bacc.Bacc):
        # Exp (softmax) and Ln (layernorm rstd) live in different default
        # activation-table sets, causing ~50 table-load thrashes (~2.7us
        # each). Restrict both to natural_log_exp_and_others (which holds
        # both) so one load serves the whole kernel.
        def insert_act_table_loads(self):
            from concourse.hw_specs import get_activation_tables
            import bass_rust as _bass_rust

            has_act = any(
                isinstance(i, mybir.InstActivation)
                for b in self.main_func.blocks
                for i in b.instructions
            )
            if not has_act:
                return
            AF2 = mybir.ActivationFunctionType
            tables = []
            for name, fns in get_activation_tables(self.m.arch).items():
                if name != "natural_log_exp_and_others":
                    fns = fns - {AF2.Exp, AF2.Ln}
                tables.append((name, fns))
            _bass_rust.insert_act_table_loads(self, tables)

    nc = _Bacc()
    mm = nc.tensor.matmul

    x_d = nc.declare_dram_parameter("x", [NDT, P, S], bf16, isOutput=False)
    wq_d = nc.declare_dram_parameter("wq", [L, P, NDT, NPAIR, P], bf16, isOutput=False)
    wk_d = nc.declare_dram_parameter("wk", [L, P, NDT, NPAIR, P], bf16, isOutput=False)
    wv_d = nc.declare_dram_parameter("wv", [L, P, NDT, H * DK], bf16, isOutput=False)
    wo_d = nc.declare_dram_parameter("wo", [L, P, NPAIR, NDT, P], bf16, isOutput=False)
    w1_d = nc.declare_dram_parameter("w1", [L, P, NDT, NFT, P], bf16, isOutput=False)
    w2_d = nc.declare_dram_parameter("w2", [L, P, NFT, NDT, P], bf16, isOutput=False)
    g1_d = nc.declare_dram_parameter("g1", [L, P, NDT], fp32, isOutput=False)
    be1_d = nc.declare_dram_parameter("be1", [L, P, NDT], fp32, isOutput=False)
    g2_d = nc.declare_dram_parameter("g2", [L, P, NDT], fp32, isOutput=False)
    be2_d = nc.declare_dram_parameter("be2", [L, P, NDT], fp32, isOutput=False)
    b1_d = nc.declare_dram_parameter("b1", [L, P, NFT], fp32, isOutput=False)
    b2_d = nc.declare_dram_parameter("b2", [L, P, NDT], fp32, isOutput=False)
    ones_d = nc.declare_dram_parameter("ones", [P, P], bf16, isOutput=False)
    e2a_d = nc.declare_dram_parameter("e2a", [1, P], fp32, isOutput=False)
    e2b_d = nc.declare_dram_parameter("e2b", [1, P], fp32, isOutput=False)
    krow_d = nc.declare_dram_parameter("krow", [1, 512], i32, isOutput=False)
    out_d = nc.declare_dram_parameter("out", [NDT, P, S], bf16, isOutput=True)

    with tile.TileContext(nc) as tc:
        from contextlib import ExitStack

        with ExitStack() as ctx:
            ec = ctx.enter_context
            ec(
                nc.allow_low_precision(
                    reason="bf16 matmul operands; fp32 PSUM accumulation"
                )
            )
            # --- SBUF pools ---
            const_p = ec(tc.tile_pool(name="const", bufs=1))
            wts_p = ec(tc.tile_pool(name="wts", bufs=1))
            w1_p = ec(tc.tile_pool(name="w1p", bufs=1))
            w2_p = ec(tc.tile_pool(name="w2p", bufs=1))
            xt_p = ec(tc.tile_pool(name="xt", bufs=8))
            qk_p = ec(tc.tile_pool(name="qk", bufs=4))
            v_p = ec(tc.tile_pool(name="v", bufs=8))
            exp_p = ec(tc.tile_pool(name="exp", bufs=6))
            ctx_p = ec(tc.tile_pool(name="ctxp", bufs=4))
            mha_p = ec(tc.tile_pool(name="mha", bufs=4))
            ff1_p = ec(tc.tile_pool(name="ff1", bufs=3))
            ysq_p = ec(tc.tile_pool(name="ysq", bufs=2))
            bcs_p = ec(tc.tile_pool(name="bcs", bufs=4))
            rows_p = ec(tc.tile_pool(name="rows", bufs=1))
            # --- PSUM pools: 4 + 2 + 2 = 8 banks ---
            # sc:  attention score pairs [128,1024] (2 banks each, x2);
            #      reused for LN broadcast pair and FFN2 accumulator pairs
            # acc: attention ctx accumulators [65,512] (x2); LN stats [1,512]
            # mm:  short-lived [128,512] matmul outputs (V/QKV/Wo/FFN1/bc)
            pp_sc = ec(tc.tile_pool(name="pp_sc", bufs=2, space="PSUM"))
            pp_acc = ec(tc.tile_pool(name="pp_acc", bufs=2, space="PSUM"))
            pp_mm = ec(tc.tile_pool(name="pp_mm", bufs=2, space="PSUM"))

            ones_full = const_p.tile([P, P], bf16)
            nc.sync.dma_start(out=ones_full, in_=ones_d[:, :])
            ones_col = ones_full[:, 0:1]
            e2a_t = const_p.tile([1, P], fp32)
            nc.sync.dma_start(out=e2a_t, in_=e2a_d[:, :])
            e2b_t = const_p.tile([1, P], fp32)
            nc.sync.dma_start(out=e2b_t, in_=e2b_d[:, :])
            krow = const_p.tile([1, 512], i32)
            nc.sync.dma_start(out=krow, in_=krow_d[:, :])
            twos = const_p.tile([1, 512], fp32)
            nc.vector.memset(twos, 2.0)
            zero_col = const_p.tile([P, 1], fp32)
            nc.vector.memset(zero_col, 0.0)
            eps_col = const_p.tile([P, 1], fp32)
            nc.vector.memset(eps_col, float(EPS))

            # layer-0 input
            xt = []
            for dt in range(NDT):
                t = xt_p.tile([P, S], bf16, tag="xt")
                nc.sync.dma_start(out=t, in_=x_d[dt])
                xt.append(t)

            for l in range(L):
                # ---------------- weight loads (bufs=1 pools serialize
                # against last use of the previous layer) -------------------
                wq_t = wts_p.tile([P, NDT, NPAIR, P], bf16, tag="wq")
                nc.sync.dma_start(out=wq_t, in_=wq_d[l])
                wk_t = wts_p.tile([P, NDT, NPAIR, P], bf16, tag="wk")
                nc.sync.dma_start(out=wk_t, in_=wk_d[l])
                wv_t = wts_p.tile([P, NDT, H * DK], bf16, tag="wv")
                nc.sync.dma_start(out=wv_t, in_=wv_d[l])
                wo_t = wts_p.tile([P, NPAIR, NDT, P], bf16, tag="wo")
                nc.sync.dma_start(out=wo_t, in_=wo_d[l])
                w1_t = w1_p.tile([P, NDT, NFT, P], bf16, tag="w1")
                nc.sync.dma_start(out=w1_t, in_=w1_d[l])
                w2_t = w2_p.tile([P, NFT, NDT, P], bf16, tag="w2")
                nc.sync.dma_start(out=w2_t, in_=w2_d[l])
                g1_t = wts_p.tile([P, NDT], fp32, tag="g1")
                nc.sync.dma_start(out=g1_t, in_=g1_d[l])
                be1_t = wts_p.tile([P, NDT], fp32, tag="be1")
                nc.sync.dma_start(out=be1_t, in_=be1_d[l])
                g2_t = wts_p.tile([P, NDT], fp32, tag="g2")
                nc.sync.dma_start(out=g2_t, in_=g2_d[l])
                be2_t = wts_p.tile([P, NDT], fp32, tag="be2")
                nc.sync.dma_start(out=be2_t, in_=be2_d[l])
                b1_t = wts_p.tile([P, NFT], fp32, tag="b1")
                nc.sync.dma_start(out=b1_t, in_=b1_d[l])
                b2_t = wts_p.tile([P, NDT], fp32, tag="b2")
                nc.sync.dma_start(out=b2_t, in_=b2_d[l])

                # ---------------- V = x @ Wv  (row-major [t, (h,dk)]) -------
                v_tiles = []
                for st in range(NST):
                    vt = v_p.tile([P, H, DK + 1], bf16, tag="v")
                    nc.vector.memset(vt[:, :, DK : DK + 1], 1.0)
                    ps = pp_mm.tile([P, 512], fp32, tag="mm")
                    for dt in range(NDT):
                        mm(
                            ps,
                            xt[dt][:, st * P : (st + 1) * P],
                            wv_t[:, dt, :],
                            start=(dt == 0),
                            stop=(dt == NDT - 1),
                        )
                    nc.vector.tensor_copy(
                        vt[:, :, 0:DK], ps.rearrange("p (h k) -> p h k", h=H)
                    )
                    v_tiles.append(vt)

                # ---------------- QKV (per head-pair, JIT) + attention ------
                # q/k tiles hold the pair: head A at partitions 0-63, head B
                # at 64-127.
                qt = [None] * NPAIR
                kt = [None] * NPAIR
                ctx_tiles = []
                for pr in range(NPAIR):
                    ch = ctx_p.tile([P, S], bf16, tag="ctx", name="ch")
                    ctx_tiles.append(ch)

                def qk_chunk(pr, w_t, tag, half):
                    # one 512-wide half of a q or k projection
                    if half == 0:
                        dst = qk_p.tile([P, S], bf16, tag=tag)
                    else:
                        dst = (qt if tag == "qt" else kt)[pr]
                    ps = pp_mm.tile([P, 512], fp32, tag="mm")
                    for dt in range(NDT):
                        mm(
                            ps,
                            w_t[:, dt, pr, :],
                            xt[dt][:, half * 512 : (half + 1) * 512],
                            start=(dt == 0),
                            stop=(dt == NDT - 1),
                        )
                    nc.vector.tensor_copy(dst[:, half * 512 : (half + 1) * 512], ps)
                    return dst

                def emit_qk(pr, chunk):
                    # chunks 0..3: q-half0, q-half1, k-half0, k-half1
                    if chunk == 0:
                        qt[pr] = qk_chunk(pr, wq_t, "qt", 0)
                    elif chunk == 1:
                        qk_chunk(pr, wq_t, "qt", 1)
                    elif chunk == 2:
                        kt[pr] = qk_chunk(pr, wk_t, "kt", 0)
                    else:
                        qk_chunk(pr, wk_t, "kt", 1)

                for chunk in range(4):
                    emit_qk(0, chunk)

                for pr in range(NPAIR):
                    hA, hB = 2 * pr, 2 * pr + 1
                    for nh in range(NH):
                        ssl = slice(nh * 512, (nh + 1) * 512)
                        psA = pp_acc.tile([DK + 1, 512], fp32, tag="acc")
                        psB = pp_acc.tile([DK + 1, 512], fp32, tag="acc")
                        for t in range(NST):
                            tsl = slice(t * P, (t + 1) * P)
                            sc = pp_sc.tile([P, 1024], fp32, tag="sc")
                            # A/B stationaries live in disjoint PE row groups
                            # (auto tile_position (0,0)/(64,0)) -> concurrent
                            mm(sc[:, 0:512], kt[pr][0:64, tsl], qt[pr][0:64, ssl])
                            mm(
                                sc[:, 512:1024],
                                kt[pr][64:128, tsl],
                                qt[pr][64:128, ssl],
                            )
                            e = exp_p.tile([P, 1024], bf16, tag="exp")
                            nc.scalar.activation(
                                e, sc, AF.Exp, bias=zero_col, scale=float(SCALE)
                            )
                            mm(
                                psA,
                                v_tiles[t][:, hA, :],
                                e[:, 0:512],
                                start=(t == 0),
                                stop=(t == NST - 1),
                            )
                            mm(
                                psB,
                                v_tiles[t][:, hB, :],
                                e[:, 512:1024],
                                start=(t == 0),
                                stop=(t == NST - 1),
                            )
                            if nh == 0 and t < 4 and pr + 1 < NPAIR:
                                emit_qk(pr + 1, t)
                        # normalize both heads: 1/denom via fast reciprocal,
                        # K=2 selector matmul broadcasts rdA to rows 0-63 and
                        # rdB to rows 64-127
                        def recip_nr(psX, tag):
                            # 1/d: bit-trick seed (K - bits(d)) on DVE, one
                            # Newton step r*(2-d*r) on GpSimd (max err 0.26%)
                            sd = rows_p.tile([1, 512], i32, tag="sd" + tag, bufs=2)
                            nc.vector.tensor_tensor(
                                out=sd, in0=krow,
                                in1=psX[DK : DK + 1].bitcast(i32),
                                op=OP.subtract,
                            )
                            dsb = rows_p.tile([1, 512], fp32, tag="d" + tag, bufs=2)
                            nc.vector.tensor_copy(dsb, psX[DK : DK + 1])
                            m = rows_p.tile([1, 512], fp32, tag="m" + tag, bufs=2)
                            nc.gpsimd.tensor_mul(m, dsb, sd.bitcast(fp32))
                            nc.gpsimd.tensor_sub(m, twos, m)
                            r = rows_p.tile([1, 512], fp32, tag="r" + tag, bufs=2)
                            nc.gpsimd.tensor_mul(r, sd.bitcast(fp32), m)
                            return r

                        rdA = recip_nr(psA, "A")
                        rdB = recip_nr(psB, "B")
                        bcs = bcs_p.tile([64, 1024], fp32, tag="bcs")
                        nc.gpsimd.partition_broadcast(bcs[:, 0:512], rdA)
                        nc.gpsimd.partition_broadcast(bcs[:, 512:1024], rdB)
                        nc.vector.tensor_mul(
                            ctx_tiles[pr][0:64, ssl], psA[0:64], bcs[:, 0:512]
                        )
                        nc.vector.tensor_mul(
                            ctx_tiles[pr][64:128, ssl], psB[0:64], bcs[:, 512:1024]
                        )

                # ---------------- Wo + residual -> y (pre-LN1) --------------
                y = []
                for mt in range(NDT):
                    yt = mha_p.tile([P, S], bf16, tag="mha")
                    y.append(yt)
                for mt in range(NDT):
                    wops = pp_sc.tile([P, 1024], fp32, tag="sc", name="wops")
                    for nh in range(NH):
                        ssl = slice(nh * 512, (nh + 1) * 512)
                        for pr in range(NPAIR):
                            mm(
                                wops[:, ssl],
                                wo_t[:, pr, mt, :],
                                ctx_tiles[pr][:, ssl],
                                start=(pr == 0),
                                stop=(pr == NPAIR - 1),
                                skip_group_check=True,
                            )
                    nc.vector.tensor_add(y[mt], wops, xt[mt])

                def layernorm(yv, g_t, be_t):
                    # in-place LN over the partition (feature) dim via
                    # ones-matmul stats; broadcast sub/mul run on GpSimd
                    for nh in range(NH):
                        ssl = slice(nh * 512, (nh + 1) * 512)
                        p1 = pp_acc.tile([1, 512], fp32, tag="acc")
                        for dt in range(NDT):
                            mm(
                                p1,
                                ones_col,
                                yv[dt][:, ssl],
                                start=(dt == 0),
                                stop=(dt == NDT - 1),
                            )
                        p2 = pp_acc.tile([1, 512], fp32, tag="acc")
                        for dt in range(NDT):
                            sq = ysq_p.tile([P, 512], bf16, tag="ysq")
                            nc.vector.tensor_mul(sq, yv[dt][:, ssl], yv[dt][:, ssl])
                            mm(
                                p2,
                                ones_col,
                                sq,
                                start=(dt == 0),
                                stop=(dt == NDT - 1),
                            )
                        mean = rows_p.tile([1, 512], fp32, tag="mean", bufs=2)
                        nc.vector.tensor_scalar_mul(mean, p1, 1.0 / D)
                        msq = rows_p.tile([1, 512], fp32, tag="msq", bufs=2)
                        nc.gpsimd.tensor_mul(msq, mean, mean)
                        var = rows_p.tile([1, 512], fp32, tag="var", bufs=2)
                        nc.vector.scalar_tensor_tensor(
                            var, p2, 1.0 / D, msq, OP.mult, OP.subtract
                        )
                        # rstd = exp(-0.5 * ln(var + eps))
                        nc.scalar.activation(var, var, AF.Ln, bias=eps_col[0:1])
                        rstd = rows_p.tile([1, 512], fp32, tag="rstd", bufs=2)
                        nc.scalar.activation(
                            rstd, var, AF.Exp, bias=zero_col[0:1], scale=-0.5
                        )
                        bcb = bcs_p.tile([P, 1024], fp32, tag="bcb")
                        nc.gpsimd.partition_broadcast(bcb[:, 0:512], mean)
                        nc.gpsimd.partition_broadcast(bcb[:, 512:1024], rstd)
                        for dt in range(NDT):
                            nc.vector.tensor_sub(
                                yv[dt][:, ssl], yv[dt][:, ssl], bcb[:, 0:512]
                            )
                            nc.gpsimd.tensor_mul(
                                yv[dt][:, ssl], yv[dt][:, ssl], bcb[:, 512:1024]
                            )
                            nc.vector.tensor_scalar(
                                out=yv[dt][:, ssl],
                                in0=yv[dt][:, ssl],
                                scalar1=g_t[:, dt : dt + 1],
                                scalar2=be_t[:, dt : dt + 1],
                                op0=OP.mult,
                                op1=OP.add,
                            )

                layernorm(y, g1_t, be1_t)  # y is now mhaT

                # ---------------- FFN ---------------------------------------
                z = []
                for mt in range(NDT):
                    zt = xt_p.tile([P, S], bf16, tag="xt")
                    z.append(zt)
                for nh in range(NH):
                    ssl = slice(nh * 512, (nh + 1) * 512)
                    ff01 = pp_sc.tile([P, 1024], fp32, tag="sc", name="ff01")
                    ff23 = pp_sc.tile([P, 1024], fp32, tag="sc", name="ff23")
                    ff2_ps = [
                        ff01[:, 0:512],
                        ff01[:, 512:1024],
                        ff23[:, 0:512],
                        ff23[:, 512:1024],
                    ]
                    for ft in range(NFT):
                        ps = pp_mm.tile([P, 512], fp32, tag="mm")
                        for dt in range(NDT):
                            mm(
                                ps,
                                w1_t[:, dt, ft, :],
                                y[dt][:, ssl],
                                start=(dt == 0),
                                stop=(dt == NDT - 1),
                            )
                        f1 = ff1_p.tile([P, 512], bf16, tag="ff1")
                        # fused bias + ReLU on the Scalar engine
                        nc.scalar.activation(
                            f1, ps, AF.Relu, bias=b1_t[:, ft : ft + 1]
                        )
                        for mt in range(NDT):
                            mm(
                                ff2_ps[mt],
                                w2_t[:, ft, mt, :],
                                f1,
                                start=(ft == 0),
                                stop=(ft == NFT - 1),
                                skip_group_check=True,
                            )
                    for mt in range(NDT):
                        nc.vector.scalar_tensor_tensor(
                            z[mt][:, ssl],
                            ff2_ps[mt],
                            b2_t[:, mt : mt + 1],
                            y[mt][:, ssl],
                            OP.add,
                            OP.add,
                        )

                layernorm(z, g2_t, be2_t)  # z is now next layer's xT
                xt = z

            for dt in range(NDT):
                nc.sync.dma_start(out=out_d[dt], in_=xt[dt])

    return nc


def _bf16(a: np.ndarray) -> np.ndarray:
    import ml_dtypes

    return np.ascontiguousarray(a).astype(ml_dtypes.bfloat16)


def _prep_weights(Wq, Wk, Wv, Wo, ln1_g, ln1_b, W1, b1, W2, b2, ln2_g, ln2_b):
    f = np.float32

    def qk_r(W):  # [L,H,D,DK] -> [L, 128, NDT, NPAIR, 128]
        return _bf16(
            W.reshape(L, NPAIR, 2, NDT, P, DK)
            .transpose(0, 4, 3, 1, 2, 5)
            .reshape(L, P, NDT, NPAIR, P)
        )

    wv_r = _bf16(
        Wv.transpose(0, 2, 1, 3)  # [L, D, H, DK]
        .reshape(L, NDT, P, H * DK)
        .transpose(0, 2, 1, 3)
        .reshape(L, P, NDT, H * DK)
    )
    # Wo packed per head-pair: rows 128*pr..128*(pr+1) as the K=128 stationary
    wo_r = _bf16(Wo.reshape(L, NPAIR, P, NDT, P).transpose(0, 2, 1, 3, 4))
    w1_r = _bf16(W1.reshape(L, NDT, P, NFT, P).transpose(0, 2, 1, 3, 4))
    w2_r = _bf16(W2.reshape(L, NFT, P, NDT, P).transpose(0, 2, 1, 3, 4))

    e2a = np.zeros((1, P), np.float32)
    e2a[0, 0:64] = 1.0
    e2b = np.zeros((1, P), np.float32)
    e2b[0, 64:128] = 1.0

    def ln_r(v, n):  # [L, n*128] -> [L, 128, n]
        return np.ascontiguousarray(v.reshape(L, n, P).transpose(0, 2, 1).astype(f))

    return {
        "wq": qk_r(Wq),
        "wk": qk_r(Wk),
        "wv": wv_r,
        "wo": wo_r,
        "w1": w1_r,
        "w2": w2_r,
        "g1": ln_r(ln1_g, NDT),
        "be1": ln_r(ln1_b, NDT),
        "g2": ln_r(ln2_g, NDT),
        "be2": ln_r(ln2_b, NDT),
        "b1": ln_r(b1, NFT),
        "b2": ln_r(b2, NDT),
        "ones": _bf16(np.ones((P, P), np.float32)),
        "e2a": e2a,
        "e2b": e2b,
        "krow": np.full((1, 512), 0x7EF311C3, dtype=np.int32),
    }


def get_nc():
    if "nc" not in _CACHE:
        nc = _build_nc()
        if not nc.is_finalized():
            nc.finalize()
        _CACHE["nc"] = nc
    return _CACHE["nc"]


def make_in_maps(**inputs):
    inputs = {k: np.asarray(v, dtype=np.float32) for k, v in inputs.items()}
    x = inputs.pop("x")
    wmap = _prep_weights(**inputs)
    in_maps = []
    for b in range(B):
        xt = _bf16(x[b].T.reshape(NDT, P, S))
        in_maps.append({"x": xt, **wmap})
    return in_maps


def kernel(**inputs) -> np.ndarray:
    from concourse.bass_utils import run_bass_kernel_spmd

    nc = get_nc()
    in_maps = make_in_maps(**inputs)
    res = run_bass_kernel_spmd(nc, in_maps, core_ids=list(range(B)))
    out = np.empty((B, S, D), dtype=np.float32)
    for b in range(B):
        out[b] = res.results[b]["out"].astype(np.float32).reshape(D, S).T
    return out


if __name__ == "__main__":
    rng = np.random.default_rng(0)
    ins = {
        "x": rng.standard_normal((B, S, D), dtype=np.float32),
        "Wq": rng.standard_normal((L, H, D, DK), dtype=np.float32) * 0.02,
        "Wk": rng.standard_normal((L, H, D, DK), dtype=np.float32) * 0.02,
        "Wv": rng.standard_normal((L, H, D, DK), dtype=np.float32) * 0.02,
        "Wo": rng.standard_normal((L, D, D), dtype=np.float32) * 0.02,
        "ln1_g": np.ones((L, D), np.float32),
        "ln1_b": np.zeros((L, D), np.float32),
        "W1": rng.standard_normal((L, D, DFF), dtype=np.float32) * 0.02,
        "b1": np.zeros((L, DFF), np.float32),
        "W2": rng.standard_normal((L, DFF, D), dtype=np.float32) * 0.02,
        "b2": np.zeros((L, D), np.float32),
        "ln2_g": np.ones((L, D), np.float32),
        "ln2_b": np.zeros((L, D), np.float32),
    }
    out = kernel(**ins)
    print(out.shape, out.dtype, np.abs(out).mean())


# revision 21
# speedup vs baseline: 1.6369x; 1.6369x over previous
"""Trainium2 Bass kernel for a 6-layer post-LN transformer encoder.

Sharding: data-parallel over batch — B=8, one batch element per NeuronCore,
no collectives.  Each core runs the full 6-layer encoder on its [S, D] slice.

Device-side layout: activations are kept feature-major ([D, S], "xT") in SBUF
so every matmul uses input-major weights as the stationary (lhsT) operand and
PE contracts over the partition dim:

  out[m, n] = sum_k lhsT[k, m] * rhs[k, n]

All matmul operands are bfloat16 (1 PE row/cycle, half the HBM/SBUF traffic of
fp32); PSUM accumulation stays fp32.  Attention is computed transposed
(scoresT[t, s]):

- scores for the two heads of a pair use K=64 stationaries at partition rows
  0-63 / 64-127, issued back-to-back into the two banks of one [128, 1024]
  PSUM tile -> the PE runs them concurrently (row-group tiling).
- one [128, 1024] Exp covers both heads of one (t, s-half) group, amortizing
  the ACT pipeline overhead; sc tiles are double-buffered so ACT stays fed.
- softmax denominators fall out of a ones-column appended to V (ctx matmul
  row 64); 1/denom via single-op reciprocal_approx_fast, broadcast to both
  head rows with one K=2 matmul against a [2,128] selector constant.
- ctx pairs are packed [head A rows 0-63 | head B rows 64-127] so Wo runs at
  K=128 (half the instructions of per-head K=64).

FFN weights stay SBUF-resident for the whole layer (loaded once, bf16), the
bias+ReLU runs fused on the Scalar engine, and LayerNorm's broadcast
subtract/multiply run on GpSimd to keep the Vector engine off the critical
path.
"""

import numpy as np

L, H, D, DK, DFF = 6, 8, 512, 64, 2048
B, S = 8, 1024
EPS = 1e-5
P = 128
NDT = D // P        # 4  d-tiles
NST = S // P        # 8  s/t-tiles
NFT = DFF // P      # 16 dff-tiles
NPAIR = H // 2      # 4  head pairs
NH = S // 512       # 2  s-halves (512-wide fp32-PSUM matmul free dim)
SCALE = 1.0 / np.sqrt(np.float32(DK))

_CACHE = {}


def _build_nc():
    import concourse.bass as bass
    import concourse.bacc as bacc
    import concourse.tile as tile
    from concourse import mybir

    fp32 = mybir.dt.float32
    fp32r = mybir.dt.float32r
    bf16 = mybir.dt.bfloat16
    i32 = mybir.dt.int32
    AF = mybir.ActivationFunctionType
    OP = mybir.AluOpType

    class _Bacc(bacc.Bacc):
        # Exp (softmax) and Ln (layernorm rstd) live in different default
        # activation-table sets, causing ~50 table-load thrashes (~2.7us
        # each). Restrict both to natural_log_exp_and_others (which holds
        # both) so one load serves the whole kernel.
        def insert_act_table_loads(self):
            from concourse.hw_specs import get_activation_tables
            import bass_rust as _bass_rust

            has_act = any(
                isinstance(i, mybir.InstActivation)
                for b in self.main_func.blocks
                for i in b.instructions
            )
            if not has_act:
                return
            AF2 = mybir.ActivationFunctionType
            tables = []
            for name, fns in get_activation_tables(self.m.arch).items():
                if name != "natural_log_exp_and_others":
                    fns = fns - {AF2.Exp, AF2.Ln}
                tables.append((name, fns))
            _bass_rust.insert_act_table_loads(self, tables)

    nc = _Bacc()
    mm = nc.tensor.matmul

    x_d = nc.declare_dram_parameter("x", [NDT, P, S], bf16, isOutput=False)
    wq_d = nc.declare_dram_parameter("wq", [L, P, NDT, NPAIR, P], bf16, isOutput=False)
    wk_d = nc.declare_dram_parameter("wk", [L, P, NDT, NPAIR, P], bf16, isOutput=False)
    wv_d = nc.declare_dram_parameter("wv", [L, P, NDT, H * DK], bf16, isOutput=False)
    wo_d = nc.declare_dram_parameter("wo", [L, P, NPAIR, NDT, P], bf16, isOutput=False)
    w1_d = nc.declare_dram_parameter("w1", [L, P, NDT, NFT, P], bf16, isOutput=False)
    w2_d = nc.declare_dram_parameter("w2", [L, P, NFT, NDT, P], bf16, isOutput=False)
    g1_d = nc.declare_dram_parameter("g1", [L, P, NDT], fp32, isOutput=False)
    be1_d = nc.declare_dram_parameter("be1", [L, P, NDT], fp32, isOutput=False)
    g2_d = nc.declare_dram_parameter("g2", [L, P, NDT], fp32, isOutput=False)
    be2_d = nc.declare_dram_parameter("be2", [L, P, NDT], fp32, isOutput=False)
    b1_d = nc.declare_dram_parameter("b1", [L, P, NFT], fp32, isOutput=False)
    b2_d = nc.declare_dram_parameter("b2", [L, P, NDT], fp32, isOutput=False)
    ones_d = nc.declare_dram_parameter("ones", [P, P], bf16, isOutput=False)
    e2a_d = nc.declare_dram_parameter("e2a", [1, P], fp32, isOutput=False)
    e2b_d = nc.declare_dram_parameter("e2b", [1, P], fp32, isOutput=False)
    krow_d = nc.declare_dram_parameter("krow", [1, 512], i32, isOutput=False)
    out_d = nc.declare_dram_parameter("out", [NDT, P, S], bf16, isOutput=True)

    with tile.TileContext(nc) as tc:
        from contextlib import ExitStack

        with ExitStack() as ctx:
            ec = ctx.enter_context
            ec(
                nc.allow_low_precision(
                    reason="bf16 matmul operands; fp32 PSUM accumulation"
                )
            )
            # --- SBUF pools ---
            const_p = ec(tc.tile_pool(name="const", bufs=1))
            wts_p = ec(tc.tile_pool(name="wts", bufs=1))
            w1_p = ec(tc.tile_pool(name="w1p", bufs=1))
            w2_p = ec(tc.tile_pool(name="w2p", bufs=1))
            xt_p = ec(tc.tile_pool(name="xt", bufs=8))
            qk_p = ec(tc.tile_pool(name="qk", bufs=4))
            v_p = ec(tc.tile_pool(name="v", bufs=8))
            exp_p = ec(tc.tile_pool(name="exp", bufs=6))
            ctx_p = ec(tc.tile_pool(name="ctxp", bufs=4))
            mha_p = ec(tc.tile_pool(name="mha", bufs=4))
            ff1_p = ec(tc.tile_pool(name="ff1", bufs=3))
            ysq_p = ec(tc.tile_pool(name="ysq", bufs=2))
            bcs_p = ec(tc.tile_pool(name="bcs", bufs=4))
            rows_p = ec(tc.tile_pool(name="rows", bufs=1))
            # --- PSUM pools: 4 + 2 + 2 = 8 banks ---
            # sc:  attention score pairs [128,1024] (2 banks each, x2);
            #      reused for LN broadcast pair and FFN2 accumulator pairs
            # acc: attention ctx accumulators [65,512] (x2); LN stats [1,512]
            # mm:  short-lived [128,512] matmul outputs (V/QKV/Wo/FFN1/bc)
            pp_sc = ec(tc.tile_pool(name="pp_sc", bufs=2, space="PSUM"))
            pp_acc = ec(tc.tile_pool(name="pp_acc", bufs=2, space="PSUM"))
            pp_mm = ec(tc.tile_pool(name="pp_mm", bufs=2, space="PSUM"))

            ones_full = const_p.tile([P, P], bf16)
            nc.sync.dma_start(out=ones_full, in_=ones_d[:, :])
            ones_col = ones_full[:, 0:1]
            e2a_t = const_p.tile([1, P], fp32)
            nc.sync.dma_start(out=e2a_t, in_=e2a_d[:, :])
            e2b_t = const_p.tile([1, P], fp32)
            nc.sync.dma_start(out=e2b_t, in_=e2b_d[:, :])
            krow = const_p.tile([1, 512], i32)
            nc.sync.dma_start(out=krow, in_=krow_d[:, :])
            twos = const_p.tile([1, 512], fp32)
            nc.vector.memset(twos, 2.0)
            zero_col = const_p.tile([P, 1], fp32)
            nc.vector.memset(zero_col, 0.0)
            eps_col = const_p.tile([P, 1], fp32)
            nc.vector.memset(eps_col, float(EPS))

            # layer-0 input
            xt = []
            for dt in range(NDT):
                t = xt_p.tile([P, S], bf16, tag="xt")
                nc.sync.dma_start(out=t, in_=x_d[dt])
                xt.append(t)

            for l in range(L):
                # ---------------- weight loads (bufs=1 pools serialize
                # against last use of the previous layer) -------------------
                wq_t = wts_p.tile([P, NDT, NPAIR, P], bf16, tag="wq")
                nc.sync.dma_start(out=wq_t, in_=wq_d[l])
                wk_t = wts_p.tile([P, NDT, NPAIR, P], bf16, tag="wk")
                nc.sync.dma_start(out=wk_t, in_=wk_d[l])
                wv_t = wts_p.tile([P, NDT, H * DK], bf16, tag="wv")
                nc.sync.dma_start(out=wv_t, in_=wv_d[l])
                wo_t = wts_p.tile([P, NPAIR, NDT, P], bf16, tag="wo")
                nc.sync.dma_start(out=wo_t, in_=wo_d[l])
                w1_t = w1_p.tile([P, NDT, NFT, P], bf16, tag="w1")
                nc.sync.dma_start(out=w1_t, in_=w1_d[l])
                w2_t = w2_p.tile([P, NFT, NDT, P], bf16, tag="w2")
                nc.sync.dma_start(out=w2_t, in_=w2_d[l])
                g1_t = wts_p.tile([P, NDT], fp32, tag="g1")
                nc.sync.dma_start(out=g1_t, in_=g1_d[l])
                be1_t = wts_p.tile([P, NDT], fp32, tag="be1")
                nc.sync.dma_start(out=be1_t, in_=be1_d[l])
                g2_t = wts_p.tile([P, NDT], fp32, tag="g2")
                nc.sync.dma_start(out=g2_t, in_=g2_d[l])
                be2_t = wts_p.tile([P, NDT], fp32, tag="be2")
                nc.sync.dma_start(out=be2_t, in_=be2_d[l])
                b1_t = wts_p.tile([P, NFT], fp32, tag="b1")
                nc.sync.dma_start(out=b1_t, in_=b1_d[l])
                b2_t = wts_p.tile([P, NDT], fp32, tag="b2")
                nc.sync.dma_start(out=b2_t, in_=b2_d[l])

                # ---------------- V = x @ Wv  (row-major [t, (h,dk)]) -------
                v_tiles = []
                for st in range(NST):
                    vt = v_p.tile([P, H, DK + 1], bf16, tag="v")
                    nc.vector.memset(vt[:, :, DK : DK + 1], 1.0)
                    ps = pp_mm.tile([P, 512], fp32, tag="mm")
                    for dt in range(NDT):
                        mm(
                            ps,
                            xt[dt][:, st * P : (st + 1) * P],
                            wv_t[:, dt, :],
                            start=(dt == 0),
                            stop=(dt == NDT - 1),
                        )
                    nc.vector.tensor_copy(
                        vt[:, :, 0:DK], ps.rearrange("p (h k) -> p h k", h=H)
                    )
                    v_tiles.append(vt)

                # ---------------- QKV (per head-pair, JIT) + attention ------
                # q/k tiles hold the pair: head A at partitions 0-63, head B
                # at 64-127.
                qt = [None] * NPAIR
                kt = [None] * NPAIR
                ctx_tiles = []
                for pr in range(NPAIR):
                    ch = ctx_p.tile([P, S], bf16, tag="ctx", name="ch")
                    ctx_tiles.append(ch)

                def qk_chunk(pr, w_t, tag, half):
                    # one 512-wide half of a q or k projection
                    if half == 0:
                        dst = qk_p.tile([P, S], bf16, tag=tag)
                    else:
                        dst = (qt if tag == "qt" else kt)[pr]
                    ps = pp_mm.tile([P, 512], fp32, tag="mm")
                    for dt in range(NDT):
                        mm(
                            ps,
                            w_t[:, dt, pr, :],
                            xt[dt][:, half * 512 : (half + 1) * 512],
                            start=(dt == 0),
                            stop=(dt == NDT - 1),
                        )
                    nc.vector.tensor_copy(dst[:, half * 512 : (half + 1) * 512], ps)
                    return dst

                def emit_qk(pr, chunk):
                    # chunks 0..3: q-half0, q-half1, k-half0, k-half1
                    if chunk == 0:
                        qt[pr] = qk_chunk(pr, wq_t, "qt", 0)
                    elif chunk == 1:
                        qk_chunk(pr, wq_t, "qt", 1)
                    elif chunk == 2:
                        kt[pr] = qk_chunk(pr, wk_t, "kt", 0)
                    else:
                        qk_chunk(pr, wk_t, "kt", 1)

                for chunk in range(4):
                    emit_qk(0, chunk)

                for pr in range(NPAIR):
                    hA, hB = 2 * pr, 2 * pr + 1
                    for nh in range(NH):
                        ssl = slice(nh * 512, (nh + 1) * 512)
                        psA = pp_acc.tile([DK + 1, 512], fp32, tag="acc")
                        psB = pp_acc.tile([DK + 1, 512], fp32, tag="acc")
                        for t in range(NST):
                            tsl = slice(t * P, (t + 1) * P)
                            sc = pp_sc.tile([P, 1024], fp32, tag="sc")
                            # A/B stationaries live in disjoint PE row groups
                            # (auto tile_position (0,0)/(64,0)) -> concurrent
                            mm(sc[:, 0:512], kt[pr][0:64, tsl], qt[pr][0:64, ssl])
                            mm(
                                sc[:, 512:1024],
                                kt[pr][64:128, tsl],
                                qt[pr][64:128, ssl],
                            )
                            e = exp_p.tile([P, 1024], bf16, tag="exp")
                            nc.scalar.activation(
                                e, sc, AF.Exp, bias=zero_col, scale=float(SCALE)
                            )
                            mm(
                                psA,
                                v_tiles[t][:, hA, :],
                                e[:, 0:512],
                                start=(t == 0),
                                stop=(t == NST - 1),
                            )
                            mm(
                                psB,
                                v_tiles[t][:, hB, :],
                                e[:, 512:1024],
                                start=(t == 0),
                                stop=(t == NST - 1),
                            )
                            if nh == 0 and t < 4 and pr + 1 < NPAIR:
                                emit_qk(pr + 1, t)
                        # normalize both heads: 1/denom via fast reciprocal,
                        # K=2 selector matmul broadcasts rdA to rows 0-63 and
                        # rdB to rows 64-127
                        rdA = rows_p.tile([1, 512], fp32, tag="rdA", bufs=3)
                        nc.vector.reciprocal(rdA, psA[DK : DK + 1])
                        rdB = rows_p.tile([1, 512], fp32, tag="rdB", bufs=3)
                        nc.vector.reciprocal(rdB, psB[DK : DK + 1])
                        bcs = bcs_p.tile([64, 1024], fp32, tag="bcs")
                        nc.gpsimd.partition_broadcast(bcs[:, 0:512], rdA)
                        nc.gpsimd.partition_broadcast(bcs[:, 512:1024], rdB)
                        nc.vector.tensor_mul(
                            ctx_tiles[pr][0:64, ssl], psA[0:64], bcs[:, 0:512]
                        )
                        nc.vector.tensor_mul(
                            ctx_tiles[pr][64:128, ssl], psB[0:64], bcs[:, 512:1024]
                        )

                # ---------------- Wo + residual -> y (pre-LN1) --------------
                y = []
                for mt in range(NDT):
                    yt = mha_p.tile([P, S], bf16, tag="mha")
                    y.append(yt)
                for mt in range(NDT):
                    wops = pp_sc.tile([P, 1024], fp32, tag="sc", name="wops")
                    for nh in range(NH):
                        ssl = slice(nh * 512, (nh + 1) * 512)
                        for pr in range(NPAIR):
                            mm(
                                wops[:, ssl],
                                wo_t[:, pr, mt, :],
                                ctx_tiles[pr][:, ssl],
                                start=(pr == 0),
                                stop=(pr == NPAIR - 1),
                                skip_group_check=True,
                            )
                    nc.vector.tensor_add(y[mt], wops, xt[mt])

                def layernorm(yv, g_t, be_t):
                    # in-place LN over the partition (feature) dim via
                    # ones-matmul stats; broadcast sub/mul run on GpSimd
                    for nh in range(NH):
                        ssl = slice(nh * 512, (nh + 1) * 512)
                        p1 = pp_acc.tile([1, 512], fp32, tag="acc")
                        for dt in range(NDT):
                            mm(
                                p1,
                                ones_col,
                                yv[dt][:, ssl],
                                start=(dt == 0),
                                stop=(dt == NDT - 1),
                            )
                        p2 = pp_acc.tile([1, 512], fp32, tag="acc")
                        for dt in range(NDT):
                            sq = ysq_p.tile([P, 512], bf16, tag="ysq")
                            nc.vector.tensor_mul(sq, yv[dt][:, ssl], yv[dt][:, ssl])
                            mm(
                                p2,
                                ones_col,
                                sq,
                                start=(dt == 0),
                                stop=(dt == NDT - 1),
                            )
                        mean = rows_p.tile([1, 512], fp32, tag="mean", bufs=2)
                        nc.vector.tensor_scalar_mul(mean, p1, 1.0 / D)
                        msq = rows_p.tile([1, 512], fp32, tag="msq", bufs=2)
                        nc.gpsimd.tensor_mul(msq, mean, mean)
                        var = rows_p.tile([1, 512], fp32, tag="var", bufs=2)
                        nc.vector.scalar_tensor_tensor(
                            var, p2, 1.0 / D, msq, OP.mult, OP.subtract
                        )
                        # rstd = exp(-0.5 * ln(var + eps))
                        nc.scalar.activation(var, var, AF.Ln, bias=eps_col[0:1])
                        rstd = rows_p.tile([1, 512], fp32, tag="rstd", bufs=2)
                        nc.scalar.activation(
                            rstd, var, AF.Exp, bias=zero_col[0:1], scale=-0.5
                        )
                        bcb = bcs_p.tile([P, 1024], fp32, tag="bcb")
                        nc.gpsimd.partition_broadcast(bcb[:, 0:512], mean)
                        nc.gpsimd.partition_broadcast(bcb[:, 512:1024], rstd)
                        for dt in range(NDT):
                            nc.vector.tensor_sub(
                                yv[dt][:, ssl], yv[dt][:, ssl], bcb[:, 0:512]
                            )
                            nc.gpsimd.tensor_mul(
                                yv[dt][:, ssl], yv[dt][:, ssl], bcb[:, 512:1024]
                            )
                            nc.vector.tensor_scalar(
                                out=yv[dt][:, ssl],
                                in0=yv[dt][:, ssl],
                                scalar1=g_t[:, dt : dt + 1],
                                scalar2=be_t[:, dt : dt + 1],
                                op0=OP.mult,
                                op1=OP.add,
                            )

                layernorm(y, g1_t, be1_t)  # y is now mhaT

                # ---------------- FFN ---------------------------------------
                z = []
                for mt in range(NDT):
                    zt = xt_p.tile([P, S], bf16, tag="xt")
                    z.append(zt)
                for nh in range(NH):
                    ssl = slice(nh * 512, (nh + 1) * 512)
                    ff01 = pp_sc.tile([P, 1024], fp32, tag="sc", name="ff01")
                    ff23 = pp_sc.tile([P, 1024], fp32, tag="sc", name="ff23")
                    ff2_ps = [
                        ff01[:, 0:512],
                        ff01[:, 512:1024],
                        ff23[:, 0:512],
                        ff23[:, 512:1024],
                    ]
                    for ft in range(NFT):
                        ps = pp_mm.tile([P, 512], fp32, tag="mm")
                        for dt in range(NDT):
                            mm(
                                ps,
                                w1_t[:, dt, ft, :],
                                y[dt][:, ssl],
                                start=(dt == 0),
                                stop=(dt == NDT - 1),
                            )
                        f1 = ff1_p.tile([P, 512], bf16, tag="ff1")
                        # fused bias + ReLU on the Scalar engine
                        nc.scalar.activation(
                            f1, ps, AF.Relu, bias=b1_t[:, ft : ft + 1]
                        )
                        for mt in range(NDT):
                            mm(
                                ff2_ps[mt],
                                w2_t[:, ft, mt, :],
                                f1,
                                start=(ft == 0),
                                stop=(ft == NFT - 1),
                                skip_group_check=True,
                            )
                    for mt in range(NDT):
                        nc.vector.scalar_tensor_tensor(
                            z[mt][:, ssl],
                            ff2_ps[mt],
                            b2_t[:, mt : mt + 1],
                            y[mt][:, ssl],
                            OP.add,
                            OP.add,
                        )

                layernorm(z, g2_t, be2_t)  # z is now next layer's xT
                xt = z

            for dt in range(NDT):
                nc.sync.dma_start(out=out_d[dt], in_=xt[dt])

    return nc


def _bf16(a: np.ndarray) -> np.ndarray:
    import ml_dtypes

    return np.ascontiguousarray(a).astype(ml_dtypes.bfloat16)


def _prep_weights(Wq, Wk, Wv, Wo, ln1_g, ln1_b, W1, b1, W2, b2, ln2_g, ln2_b):
    f = np.float32

    def qk_r(W):  # [L,H,D,DK] -> [L, 128, NDT, NPAIR, 128]
        return _bf16(
            W.reshape(L, NPAIR, 2, NDT, P, DK)
            .transpose(0, 4, 3, 1, 2, 5)
            .reshape(L, P, NDT, NPAIR, P)
        )

    wv_r = _bf16(
        Wv.transpose(0, 2, 1, 3)  # [L, D, H, DK]
        .reshape(L, NDT, P, H * DK)
        .transpose(0, 2, 1, 3)
        .reshape(L, P, NDT, H * DK)
    )
    # Wo packed per head-pair: rows 128*pr..128*(pr+1) as the K=128 stationary
    wo_r = _bf16(Wo.reshape(L, NPAIR, P, NDT, P).transpose(0, 2, 1, 3, 4))
    w1_r = _bf16(W1.reshape(L, NDT, P, NFT, P).transpose(0, 2, 1, 3, 4))
    w2_r = _bf16(W2.reshape(L, NFT, P, NDT, P).transpose(0, 2, 1, 3, 4))

    e2a = np.zeros((1, P), np.float32)
    e2a[0, 0:64] = 1.0
    e2b = np.zeros((1, P), np.float32)
    e2b[0, 64:128] = 1.0

    def ln_r(v, n):  # [L, n*128] -> [L, 128, n]
        return np.ascontiguousarray(v.reshape(L, n, P).transpose(0, 2, 1).astype(f))

    return {
        "wq": qk_r(Wq),
        "wk": qk_r(Wk),
        "wv": wv_r,
        "wo": wo_r,
        "w1": w1_r,
        "w2": w2_r,
        "g1": ln_r(ln1_g, NDT),
        "be1": ln_r(ln1_b, NDT),
        "g2": ln_r(ln2_g, NDT),
        "be2": ln_r(ln2_b, NDT),
        "b1": ln_r(b1, NFT),
        "b2": ln_r(b2, NDT),
        "ones": _bf16(np.ones((P, P), np.float32)),
        "e2a": e2a,
        "e2b": e2b,
        "krow": np.full((1, 512), 0x7EF311C3, dtype=np.int32),
    }


def get_nc():
    if "nc" not in _CACHE:
        nc = _build_nc()
        if not nc.is_finalized():
            nc.finalize()
        _CACHE["nc"] = nc
    return _CACHE["nc"]


def make_in_maps(**inputs):
    inputs = {k: np.asarray(v, dtype=np.float32) for k, v in inputs.items()}
    x = inputs.pop("x")
    wmap = _prep_weights(**inputs)
    in_maps = []
    for b in range(B):
        xt = _bf16(x[b].T.reshape(NDT, P, S))
        in_maps.append({"x": xt, **wmap})
    return in_maps


def kernel(**inputs) -> np.ndarray:
    from concourse.bass_utils import run_bass_kernel_spmd

    nc = get_nc()
    in_maps = make_in_maps(**inputs)
    res = run_bass_kernel_spmd(nc, in_maps, core_ids=list(range(B)))
    out = np.empty((B, S, D), dtype=np.float32)
    for b in range(B):
        out[b] = res.results[b]["out"].astype(np.float32).reshape(D, S).T
    return out


if __name__ == "__main__":
    rng = np.random.default_rng(0)
    ins = {
        "x": rng.standard_normal((B, S, D), dtype=np.float32),
        "Wq": rng.standard_normal((L, H, D, DK), dtype=np.float32) * 0.02,
        "Wk": rng.standard_normal((L, H, D, DK), dtype=np.float32) * 0.02,
        "Wv": rng.standard_normal((L, H, D, DK), dtype=np.float32) * 0.02,
        "Wo": rng.standard_normal((L, D, D), dtype=np.float32) * 0.02,
        "ln1_g": np.ones((L, D), np.float32),
        "ln1_b": np.zeros((L, D), np.float32),
        "W1": rng.standard_normal((L, D, DFF), dtype=np.float32) * 0.02,
        "b1": np.zeros((L, DFF), np.float32),
        "W2": rng.standard_normal((L, DFF, D), dtype=np.float32) * 0.02,
        "b2": np.zeros((L, D), np.float32),
        "ln2_g": np.ones((L, D), np.float32),
        "ln2_b": np.zeros((L, D), np.float32),
    }
    out = kernel(**ins)
    print(out.shape, out.dtype, np.abs(out).mean())
